# revision 12
# baseline (speedup 1.0000x reference)
"""Distributed Trainium2 kernel for AdaptiveSimpleGCNConv.

Math: out = D^{-1/2} (A_set + I) D^{-1/2} @ x @ W.T + b
  A_set: dense 0/1 adjacency from edge_index (duplicates collapse), N=8192.

Strategy (8 NeuronCores, 1D row partition of nodes):
  - Host: dedup edges, compute degree/d=1/sqrt(deg), fold the column scale
    into x' = d*x. Quantize x' to fp8 (hi) plus a 64x-scaled fp8 residual
    (lo). Permute the COLUMN (source-node) space so the columns with the
    largest quantization-error energy come first; the lo correction is only
    applied to the first NLO=36 of 64 column-chunks (~72% of the error
    energy), which keeps the PE cost at 1.56x a single fp8 pass while the
    final relative error stays ~1.3e-2 (< 2e-2 gate).
  - Device k: stream adjacency supertiles (fp8, values 0/1/2 exact); for
    each chunk-pair one fp8 DoubleRow matmul (2 contraction chunks per
    instruction, 2 elem/cycle) accumulates y_hi per 512-row window, plus a
    second DoubleRow matmul into y_lo for the corrected chunk range.
    Epilogue: cast y_hi/y_lo to bf16, out = (y_hi@W.T + y_lo@(W/64).T)*d + b
    via two accumulating PSUM matmuls, stored bf16 in a [part, group, feat]
    layout the host untangles (and casts back to fp32).
  - No collectives: x planes are replicated to every core by the host.
"""

import sys

sys.path.insert(0, "/opt/trn_rl_repo")

import numpy as np
import ml_dtypes

N = 8192
D = 128
NCORES = 8
RPC = N // NCORES   # 1024 rows per core
NCHUNK = N // 128   # 64 contraction chunks
NPAIR = NCHUNK // 2  # 32 DoubleRow chunk-pairs
NLO = 36            # chunks receiving the lo correction (supertile-aligned)
NLO_PAIR = NLO // 2
NWIN = RPC // 512   # 2 row windows per core
SUPER = 4           # chunks per adjacency supertile DMA
NSUPER = NCHUNK // SUPER
XPIECE = 8          # chunks per x-plane DMA piece
S_LO = 64.0         # scale for the lo fp8 plane
BF16 = ml_dtypes.bfloat16
FP8 = ml_dtypes.float8_e4m3fn

_CACHE = {}


def _build_nc():
    from concourse import bacc, bass, tile, mybir

    adt = mybir.dt.float8e4

    nc = bacc.Bacc("TRN2", target_bir_lowering=False, debug=False,
                   num_devices=NCORES)

    adjt_ext = nc.declare_dram_parameter(
        "adjT", [128, NCHUNK, RPC], adt, isOutput=False)
    xh_ext = nc.declare_dram_parameter(
        "xh", [128, NCHUNK, D], adt, isOutput=False)
    xl_ext = nc.declare_dram_parameter(
        "xl", [128, NLO, D], adt, isOutput=False)
    wt_ext = nc.declare_dram_parameter(
        "wT", [D, D], mybir.dt.bfloat16, isOutput=False)
    wt64_ext = nc.declare_dram_parameter(
        "wT64", [D, D], mybir.dt.bfloat16, isOutput=False)
    bb_ext = nc.declare_dram_parameter(
        "bb", [128, D], mybir.dt.float32, isOutput=False)
    dr_ext = nc.declare_dram_parameter(
        "dr", [128, RPC // 128], mybir.dt.float32, isOutput=False)
    out_ext = nc.declare_dram_parameter(
        "out", [128, RPC // 128, D], mybir.dt.bfloat16, isOutput=True)

    DR = mybir.MatmulPerfMode.DoubleRow
    NXH = NCHUNK // XPIECE          # 8 hi pieces
    NXL = (NLO + XPIECE - 1) // XPIECE  # 5 lo pieces (last is short)

    with tile.TileContext(nc) as tc:
        with (
            tc.tile_pool(name="const", bufs=1) as constp,
            tc.tile_pool(name="adj", bufs=12) as adjp,
            tc.tile_pool(name="yt", bufs=2) as ytp,
            tc.tile_pool(name="ot", bufs=2) as otp,
            tc.tile_pool(name="ps_y", bufs=1, space=bass.MemorySpace.PSUM) as psy,
            tc.tile_pool(name="ps_o", bufs=2, space=bass.MemorySpace.PSUM) as pso,
        ):
            # x pieces are interleaved with adjacency supertiles on the SAME
            # (sync) queue so each piece lands just before the supertile that
            # consumes it -- see the supertile loop below.
            xh = [constp.tile([128, XPIECE, D], adt, name=f"xh{i}",
                              tag=f"xh{i}") for i in range(NXH)]
            xl = []
            for i in range(NXH):
                lo_n = min(XPIECE, max(0, NLO - i * XPIECE))
                if lo_n > 0:
                    xl.append(constp.tile([128, lo_n, D], adt,
                                          name=f"xl{i}", tag=f"xl{i}"))

            def load_xpiece(i):
                nc.sync.dma_start(
                    out=xh[i][:],
                    in_=xh_ext[:, i * XPIECE:(i + 1) * XPIECE, :])
                lo_n = min(XPIECE, max(0, NLO - i * XPIECE))
                if lo_n > 0:
                    nc.sync.dma_start(
                        out=xl[i][:],
                        in_=xl_ext[:, i * XPIECE:i * XPIECE + lo_n, :])

            wt = constp.tile([D, D], mybir.dt.bfloat16, name="wt")
            nc.scalar.dma_start(out=wt[:], in_=wt_ext[:])
            wt64 = constp.tile([D, D], mybir.dt.bfloat16, name="wt64")
            nc.scalar.dma_start(out=wt64[:], in_=wt64_ext[:])
            bb = constp.tile([128, D], mybir.dt.float32, name="bb")
            nc.scalar.dma_start(out=bb[:], in_=bb_ext[:])
            dr = constp.tile([128, RPC // 128], mybir.dt.float32, name="dr")
            nc.scalar.dma_start(out=dr[:], in_=dr_ext[:])

            ps_hi = [psy.tile([128, 512], mybir.dt.float32, tag=f"pshi{w}",
                              name=f"ps_hi{w}") for w in range(NWIN)]
            ps_lo = [psy.tile([128, 512], mybir.dt.float32, tag=f"pslo{w}",
                              name=f"ps_lo{w}") for w in range(NWIN)]

            def xslice(xs_list, q):
                # chunk-pair q -> [128, 2, D] slice of the owning piece
                c0 = 2 * q
                i, o = c0 // XPIECE, c0 % XPIECE
                return xs_list[i][:, o:o + 2, :]

            def mm(q, j, w, at):
                cs = slice(2 * j, 2 * j + 2)
                ws = slice(w * 512, (w + 1) * 512)
                nc.tensor.matmul(
                    ps_hi[w][:],
                    lhsT=xslice(xh, q),
                    rhs=at[:, cs, ws],
                    start=(q == 0),
                    stop=(q == NPAIR - 1),
                    perf_mode=DR,
                )
                if q < NLO_PAIR:
                    nc.tensor.matmul(
                        ps_lo[w][:],
                        lhsT=xslice(xl, q),
                        rhs=at[:, cs, ws],
                        start=(q == 0),
                        stop=(q == NLO_PAIR - 1),
                        perf_mode=DR,
                    )

            for s in range(NSUPER):
                if s % 2 == 0:
                    load_xpiece(s // 2)
                at = adjp.tile([128, SUPER, RPC], adt, tag="adjtile")
                nc.sync.dma_start(
                    out=at[:], in_=adjt_ext[:, s * SUPER:(s + 1) * SUPER, :])
                if s < NSUPER - 1:
                    for j in range(SUPER // 2):
                        for w in range(NWIN):
                            mm(s * 2 + j, j, w, at)
                else:
                    # last supertile window-major: window 0 finishes early so
                    # its epilogue overlaps window 1's tail matmuls
                    for w in range(NWIN):
                        for j in range(SUPER // 2):
                            mm(s * 2 + j, j, w, at)

            for w in range(NWIN):
                yh = ytp.tile([128, 512], mybir.dt.bfloat16, tag="yh")
                nc.vector.tensor_copy(yh[:], ps_hi[w][:])
                yl = ytp.tile([128, 512], mybir.dt.bfloat16, tag="yl")
                nc.vector.tensor_copy(yl[:], ps_lo[w][:])
                ot = otp.tile([128, 4, D], mybir.dt.bfloat16, tag="outtile")
                for m in range(4):
                    g = w * 4 + m
                    sl = slice(m * 128, (m + 1) * 128)
                    ps_o = pso.tile([128, D], mybir.dt.float32)
                    nc.tensor.matmul(
                        ps_o[:],
                        lhsT=yh[:, sl],
                        rhs=wt[:],
                        start=True,
                        stop=False,
                    )
                    nc.tensor.matmul(
                        ps_o[:],
                        lhsT=yl[:, sl],
                        rhs=wt64[:],
                        start=False,
                        stop=True,
                    )
                    nc.vector.scalar_tensor_tensor(
                        out=ot[:, m, :],
                        in0=ps_o[:],
                        scalar=dr[:, g:g + 1],
                        in1=bb[:],
                        op0=mybir.AluOpType.mult,
                        op1=mybir.AluOpType.add,
                    )
                nc.scalar.dma_start(out=out_ext[:, w * 4:(w + 1) * 4, :],
                                    in_=ot[:])
    nc.compile()
    return nc


def _host_prep(x, edge_index, W, b):
    r = np.asarray(edge_index[0]).astype(np.int64)
    c = np.asarray(edge_index[1]).astype(np.int64)
    uniq = np.unique(r * N + c)
    r_u = uniq // N
    c_u = uniq % N

    degree = np.bincount(r_u, minlength=N).astype(np.float64) + 1.0
    d = (1.0 / np.sqrt(degree)).astype(np.float32)

    xp = np.asarray(x, dtype=np.float32) * d[:, None]
    xh8 = xp.astype(FP8)
    lo = xp - xh8.astype(np.float32)
    xl8 = (lo * S_LO).astype(FP8)

    # permute the column space so the columns with the largest fp8
    # quantization-error energy land in the corrected chunk range [0, NLO)
    order = np.argsort(-(lo * lo).sum(axis=1), kind="stable")
    P = np.empty(N, dtype=np.int64)
    P[order] = np.arange(N)

    def to_chunks(a, nchunk):
        return np.ascontiguousarray(
            a.reshape(nchunk, 128, D).transpose(1, 0, 2))  # [128, chunk, feat]

    xh_c = to_chunks(xh8[order], NCHUNK)
    xl_c = to_chunks(xl8[order[:NLO * 128]], NLO)

    wt = np.ascontiguousarray(np.asarray(W, dtype=np.float32).T).astype(BF16)
    wt64 = np.ascontiguousarray(
        np.asarray(W, dtype=np.float32).T / S_LO).astype(BF16)
    bb = np.ascontiguousarray(
        np.tile(np.asarray(b, dtype=np.float32)[None, :], (128, 1)))

    in_maps = []
    for k in range(NCORES):
        mask = (r_u // RPC) == k
        rr = r_u[mask] - k * RPC  # local row in [0, RPC)
        cs = P[c_u[mask]]         # permuted global col in [0, N)
        adjt = np.zeros((128, NCHUNK, RPC), dtype=FP8)
        # adjt[p, cc, q] corresponds to adj[row = q (local), col = cc*128+p]
        adjt[cs & 127, cs >> 7, rr] = 1.0
        jj = np.arange(RPC)
        ii = P[k * RPC + jj]  # permuted diag index -> column
        adjt[ii & 127, ii >> 7, jj] += np.ones(RPC, dtype=FP8)
        dr = np.ascontiguousarray(
            d[k * RPC:(k + 1) * RPC].reshape(RPC // 128, 128).T)
        in_maps.append({"adjT": adjt, "xh": xh_c, "xl": xl_c,
                        "wT": wt, "wT64": wt64, "bb": bb, "dr": dr})
    return in_maps


def _gather(res):
    outs = []
    for k in range(NCORES):
        o = np.asarray(res.results[k]["out"])  # [128, RPC//128, D] bf16
        outs.append(o.transpose(1, 0, 2).reshape(RPC, D))
    return np.ascontiguousarray(np.concatenate(outs, axis=0).astype(np.float32))


def kernel(x, edge_index, W, b):
    from concourse.bass_utils import run_bass_kernel_spmd

    in_maps = _host_prep(x, edge_index, W, b)
    if "nc" not in _CACHE:
        _CACHE["nc"] = _build_nc()
    nc = _CACHE["nc"]
    res = run_bass_kernel_spmd(nc, in_maps, core_ids=list(range(NCORES)))
    return _gather(res)


if __name__ == "__main__":
    rng = np.random.default_rng(0)
    x = rng.standard_normal((N, D), dtype=np.float32)
    ei = rng.integers(0, N, size=(2, 262144)).astype(np.int64)
    W = rng.standard_normal((D, D), dtype=np.float32) / np.sqrt(D)
    b = rng.standard_normal(D, dtype=np.float32) * 0.01
    out = kernel(x=x, edge_index=ei, W=W, b=b)
    print(out.shape, out.dtype, float(np.abs(out).mean()))
